# revision 1
# baseline (speedup 1.0000x reference)
"""Cost-volume kernel (nn_CostVolume) for Trainium2, 8 NeuronCores.

out[b, i, h, w] = mean_c feat1[b, c, h, w] * feat2[b, c, h, w + i - 4]
(feat2 zero-padded along width), inputs (8, 256, 96, 320) fp32,
output (8, 9, 96, 320) fp32.

Strategy
--------
Data-parallel over B: core b handles batch b (communication-free).

Per core, for each (h, 64-wide w-block) the 9 shifted channel-dot-products
are computed on the TensorEngine as a banded correlation matmul:

    band[p, n] = sum_c f1[c, w0+p] * f2[c, w0-4+n],   p in [0,64), n in [0,72)

with the C=256 contraction split into two PSUM-accumulated K=128 matmuls.
The 9 useful diagonals band[p, p+i] cannot be extracted by any lockstep
engine (per-partition-varying offsets), so bands are staged to an HBM
scratch buffer where the diagonal of a row-major matrix is a plain strided
access: one DMA per (w-block, shift) gathers out[i, :, w-block] with read
stride (row_len + 1) and contiguous writes.
"""

import numpy as np

import concourse.bacc as bacc
import concourse.bass as bass
import concourse.tile as tile
from concourse import mybir
from concourse.bass_utils import run_bass_kernel_spmd

B, C, H, W = 8, 256, 96, 320
D = 4
NS = 2 * D + 1  # 9 shifts
P = 128  # partitions per c-block
M = 64  # w-block size
NB = W // M  # 5 w-blocks
NBAND = M + 2 * D  # 72 band columns
NH = 8  # h rows per chunk
NCHUNK = H // NH  # 12
WP = W + 2 * D  # padded feat2 row

F32 = mybir.dt.float32

_cache: dict = {}


def _build():
    nc = bacc.Bacc("TRN2", target_bir_lowering=False, debug=False, num_devices=B)
    f1 = nc.dram_tensor("f1", (C, H, W), F32, kind="ExternalInput")
    f2 = nc.dram_tensor("f2", (C, H, W), F32, kind="ExternalInput")
    out = nc.dram_tensor("out", (NS, H, W), F32, kind="ExternalOutput")

    with tile.TileContext(nc) as tc:
        with (
            tc.tile_pool(name="feat", bufs=2) as fpool,
            tc.tile_pool(name="band", bufs=2) as bpool,
            tc.tile_pool(name="ps", bufs=8, space="PSUM") as pspool,
            tc.tile_pool(name="scratch", bufs=1, space="DRAM") as dpool,
        ):
            # scratch layout [h, p, blk*NBAND]; diag of each 72-col band row
            # block is then stride-(360+1) in (p, n).
            scratch = dpool.tile([H, M, NB * NBAND], F32)
            sc_h = M * NB * NBAND  # 23040 elements per h row
            sc_p = NB * NBAND  # 360 elements per p row

            for chunk in range(NCHUNK):
                h0 = chunk * NH
                f1t = []
                f2t = []
                for cb in range(2):
                    t1 = fpool.tile([P, NH, W], F32, tag=f"f1_{cb}")
                    nc.sync.dma_start(
                        out=t1, in_=f1.ap()[cb * P : (cb + 1) * P, h0 : h0 + NH, :]
                    )
                    f1t.append(t1)
                    t2 = fpool.tile([P, NH, WP], F32, tag=f"f2_{cb}")
                    nc.vector.memset(t2[:, :, 0:D], 0.0)
                    nc.vector.memset(t2[:, :, D + W : WP], 0.0)
                    nc.sync.dma_start(
                        out=t2[:, :, D : D + W],
                        in_=f2.ap()[cb * P : (cb + 1) * P, h0 : h0 + NH, :],
                    )
                    f2t.append(t2)

                band = bpool.tile([M, NH, NB * NBAND], F32, tag="band")
                for hl in range(NH):
                    ps = pspool.tile([M, NB * NBAND], F32, tag="ps")
                    for blk in range(NB):
                        w0 = blk * M
                        for cb in range(2):
                            nc.tensor.matmul(
                                ps[:, blk * NBAND : (blk + 1) * NBAND],
                                f1t[cb][:, hl, w0 : w0 + M],
                                f2t[cb][:, hl, w0 : w0 + NBAND],
                                start=(cb == 0),
                                stop=(cb == 1),
                            )
                    nc.scalar.activation(
                        band[:, hl, :],
                        ps[:],
                        mybir.ActivationFunctionType.Copy,
                        scale=1.0 / C,
                    )

                # scratch[h0:h0+NH] viewed as (p, h, f) to match band (p, hl, f)
                dst = scratch[h0 : h0 + NH, :, :].transpose([1, 0, 2])
                nc.sync.dma_start(out=dst, in_=band)

            # Diagonal gathers: scratch -> out, one DMA per (w-block, shift).
            with nc.allow_non_contiguous_dma("banded diagonal gather"):
                for blk in range(NB):
                    for i in range(NS):
                        src = bass.AP(
                            tensor=scratch.tensor,
                            offset=scratch.offset + blk * NBAND + i,
                            ap=[[sc_h, H], [sc_p + 1, M]],
                        )
                        dst = out.ap()[i, :, blk * M : (blk + 1) * M]
                        nc.sync.dma_start(out=dst, in_=src)

    nc.compile()
    return nc


def kernel(feat1: np.ndarray, feat2: np.ndarray) -> np.ndarray:
    if "nc" not in _cache:
        _cache["nc"] = _build()
    nc = _cache["nc"]
    feat1 = np.ascontiguousarray(feat1, dtype=np.float32)
    feat2 = np.ascontiguousarray(feat2, dtype=np.float32)
    in_maps = [{"f1": feat1[b], "f2": feat2[b]} for b in range(B)]
    res = run_bass_kernel_spmd(nc, in_maps, core_ids=list(range(B)))
    return np.stack([res.results[b]["out"] for b in range(B)], axis=0)


# revision 3
# speedup vs baseline: 35.8652x; 35.8652x over previous
"""Cost-volume kernel (nn_CostVolume) for Trainium2, 8 NeuronCores.

out[b, i, h, w] = mean_c feat1[b, c, h, w] * feat2[b, c, h, w + i - 4]
(feat2 zero-padded along width), inputs (8, 256, 96, 320) fp32,
output (8, 9, 96, 320) fp32.

Strategy
--------
Data-parallel over B: core b handles batch b (communication-free).

Per core, for each (h, 64-wide w-block) the 9 shifted channel-dot-products
are computed on the TensorEngine as a banded correlation matmul:

    band[p, n] = sum_c f1[c, w0+p] * f2[c, w0-4+n],   p in [0,64), n in [0,72)

with the C=256 contraction split into two PSUM-accumulated K=128 matmuls.
The 9 useful diagonals band[p, p+i] cannot be extracted by any lockstep
engine (per-partition-varying offsets), so bands are staged to an HBM
scratch buffer where the diagonal of a row-major matrix is a plain strided
access: one DMA per (w-block, shift) gathers out[i, :, w-block] with read
stride (row_len + 1) and contiguous writes.
"""

import numpy as np

import concourse.bacc as bacc
import concourse.bass as bass
import concourse.tile as tile
from concourse import mybir
from concourse.bass_utils import run_bass_kernel_spmd

B, C, H, W = 8, 256, 96, 320
D = 4
NS = 2 * D + 1  # 9 shifts
P = 128  # partitions per c-block
M = 64  # w-block size
NB = W // M  # 5 w-blocks
NBAND = M + 2 * D  # 72 band columns
NH = 8  # h rows per chunk
NCHUNK = H // NH  # 12
WP = W + 2 * D  # padded feat2 row

F32 = mybir.dt.float32

_cache: dict = {}


def _build(reps: int = 1):
    nc = bacc.Bacc("TRN2", target_bir_lowering=False, debug=False, num_devices=B)
    f1 = nc.dram_tensor("f1", (C, H, W), F32, kind="ExternalInput")
    f2 = nc.dram_tensor("f2", (C, H, W), F32, kind="ExternalInput")
    out = nc.dram_tensor("out", (NS, H, W), F32, kind="ExternalOutput")

    with tile.TileContext(nc) as tc:
        with (
            tc.tile_pool(name="feat", bufs=2) as fpool,
            tc.tile_pool(name="band", bufs=2) as bpool,
            tc.tile_pool(name="ps", bufs=8, space="PSUM") as pspool,
            tc.tile_pool(name="scratch", bufs=1, space="DRAM") as dpool,
        ):
            for _rep in range(reps):
                _body(nc, tc, fpool, bpool, pspool, dpool, f1, f2, out)

    nc.compile()
    return nc


def _body(nc, tc, fpool, bpool, pspool, dpool, f1, f2, out):
    if True:
        if True:
            # scratch layout [h, p, blk*NBAND]; diag of each 72-col band row
            # block is then stride-(360+1) in (p, n).
            scratch = dpool.tile([H, M, NB * NBAND], F32)
            sc_h = M * NB * NBAND  # 23040 elements per h row
            sc_p = NB * NBAND  # 360 elements per p row

            for chunk in range(NCHUNK):
                h0 = chunk * NH
                f1t = []
                f2t = []
                for cb in range(2):
                    t1 = fpool.tile([P, NH, W], F32, tag=f"f1_{cb}")
                    nc.sync.dma_start(
                        out=t1, in_=f1.ap()[cb * P : (cb + 1) * P, h0 : h0 + NH, :]
                    )
                    f1t.append(t1)
                    t2 = fpool.tile([P, NH, WP], F32, tag=f"f2_{cb}")
                    nc.vector.memset(t2[:, :, 0:D], 0.0)
                    nc.vector.memset(t2[:, :, D + W : WP], 0.0)
                    nc.sync.dma_start(
                        out=t2[:, :, D : D + W],
                        in_=f2.ap()[cb * P : (cb + 1) * P, h0 : h0 + NH, :],
                    )
                    f2t.append(t2)

                band = bpool.tile([M, NH, NB * NBAND], F32, tag="band")
                for hl in range(NH):
                    ps = pspool.tile([M, NB * NBAND], F32, tag="ps")
                    for blk in range(NB):
                        w0 = blk * M
                        for cb in range(2):
                            nc.tensor.matmul(
                                ps[:, blk * NBAND : (blk + 1) * NBAND],
                                f1t[cb][:, hl, w0 : w0 + M],
                                f2t[cb][:, hl, w0 : w0 + NBAND],
                                start=(cb == 0),
                                stop=(cb == 1),
                            )
                    nc.scalar.activation(
                        band[:, hl, :],
                        ps[:],
                        mybir.ActivationFunctionType.Copy,
                        scale=1.0 / C,
                    )

                # scratch[h0:h0+NH] viewed as (p, h, f) to match band (p, hl, f)
                dst = scratch[h0 : h0 + NH, :, :].transpose([1, 0, 2])
                nc.sync.dma_start(out=dst, in_=band)

            # Diagonal gathers: scratch -> out, one DMA per (w-block, shift).
            with nc.allow_non_contiguous_dma("banded diagonal gather"):
                for blk in range(NB):
                    for i in range(NS):
                        src = bass.AP(
                            tensor=scratch.tensor,
                            offset=scratch.offset + blk * NBAND + i,
                            ap=[[sc_h, H], [sc_p + 1, M]],
                        )
                        dst = out.ap()[i, :, blk * M : (blk + 1) * M]
                        nc.sync.dma_start(out=dst, in_=src)


def kernel(feat1: np.ndarray, feat2: np.ndarray) -> np.ndarray:
    if "nc" not in _cache:
        _cache["nc"] = _build()
    nc = _cache["nc"]
    feat1 = np.ascontiguousarray(feat1, dtype=np.float32)
    feat2 = np.ascontiguousarray(feat2, dtype=np.float32)
    in_maps = [{"f1": feat1[b], "f2": feat2[b]} for b in range(B)]
    res = run_bass_kernel_spmd(nc, in_maps, core_ids=list(range(B)))
    return np.stack([res.results[b]["out"] for b in range(B)], axis=0)


# revision 9
# speedup vs baseline: 427.9525x; 11.9322x over previous
"""Cost-volume kernel (nn_CostVolume) for Trainium2, 8 NeuronCores.

out[b, i, h, w] = mean_c feat1[b, c, h, w] * feat2[b, c, h, w + i - 4]
(feat2 zero-padded along width), inputs (8, 256, 96, 320) fp32,
output (8, 9, 96, 320) fp32.

Strategy
--------
Data-parallel over B: core b handles batch b (communication-free).

Per core, for each (h, 64-wide w-block) the 9 shifted channel-dot-products
are computed on the TensorEngine as a banded correlation matmul:

    band[p, n] = sum_c f1[c, w0+p] * f2[c, w0-4+n],   p in [0,64), n in [0,72)

with the C=256 contraction split into two PSUM-accumulated K=128 matmuls.
The 9 useful diagonals band[p, p+i] cannot be extracted by any lockstep
engine (per-partition-varying offsets).  Instead the bands are kept
SBUF-resident for all 96 h rows in (w-block, n, h) layout, written once to
an HBM scratch buffer (flat-addressed), and the diagonals are gathered by
DMA as 96-element contiguous h-runs with read stride (row+1) in (p, n).
The gathered [w, h] tiles are transposed to [h, w] on the TensorEngine and
written out contiguously.
"""

import numpy as np

import concourse.bacc as bacc
import concourse.bass as bass
import concourse.tile as tile
from concourse import mybir
from concourse.bass_utils import run_bass_kernel_spmd
from concourse.masks import make_identity

B, C, H, W = 8, 256, 96, 320
D = 4
NS = 2 * D + 1  # 9 shifts
P = 128  # partitions per c-block
M = 64  # w-block size
NB = W // M  # 5 w-blocks
NBAND = M + 2 * D  # 72 band columns
NH = 4  # h rows per feature chunk
NCHUNK = H // NH  # 24
WP = W + 2 * D  # padded feat2 row

F32 = mybir.dt.float32

_cache: dict = {}


def _build(reps: int = 1, skip_gather: bool = False, skip_compute: bool = False):
    nc = bacc.Bacc("TRN2", target_bir_lowering=False, debug=False, num_devices=B)
    f1 = nc.dram_tensor("f1", (C, H, W), F32, kind="ExternalInput")
    f2 = nc.dram_tensor("f2", (C, H, W), F32, kind="ExternalInput")
    out = nc.dram_tensor("out", (NS, H, W), F32, kind="ExternalOutput")

    with tile.TileContext(nc) as tc:
        with (
            tc.tile_pool(name="consts", bufs=1) as cpool,
            tc.tile_pool(name="feat", bufs=2) as fpool,
            tc.tile_pool(name="band", bufs=1) as bpool,
            tc.tile_pool(name="gat", bufs=4) as gpool,
            tc.tile_pool(name="osb", bufs=2) as opool,
            tc.tile_pool(name="ps", bufs=6, space="PSUM") as pspool,
            tc.tile_pool(name="scratch", bufs=1, space="DRAM") as dpool,
        ):
            ident = cpool.tile([M, M], F32)
            make_identity(nc, ident)
            for _rep in range(reps):
                _body(
                    nc, tc, fpool, bpool, gpool, opool, pspool, dpool, ident,
                    f1, f2, out,
                    skip_gather=skip_gather, skip_compute=skip_compute,
                )

    nc.compile()
    return nc


def _body(nc, tc, fpool, bpool, gpool, opool, pspool, dpool, ident, f1, f2, out,
          skip_gather=False, skip_compute=False):
    # SBUF-resident bands for the full image: [p, blk, n, h].
    band = bpool.tile([M, NB, NBAND, H], F32, tag="band")

    for chunk in range(NCHUNK):
        h0 = chunk * NH
        f1t = []
        f2t = []
        for cb in range(2):
            t1 = fpool.tile([P, NH, W], F32, tag=f"f1_{cb}")
            nc.sync.dma_start(
                out=t1, in_=f1.ap()[cb * P : (cb + 1) * P, h0 : h0 + NH, :]
            )
            f1t.append(t1)
            t2 = fpool.tile([P, NH, WP], F32, tag=f"f2_{cb}")
            nc.vector.memset(t2[:, :, 0:D], 0.0)
            nc.vector.memset(t2[:, :, D + W : WP], 0.0)
            nc.sync.dma_start(
                out=t2[:, :, D : D + W],
                in_=f2.ap()[cb * P : (cb + 1) * P, h0 : h0 + NH, :],
            )
            f2t.append(t2)

        if skip_compute:
            continue
        for hl in range(NH):
            ps = pspool.tile([M, NB * NBAND], F32, tag="ps")
            for blk in range(NB):
                w0 = blk * M
                for cb in range(2):
                    nc.tensor.matmul(
                        ps[:, blk * NBAND : (blk + 1) * NBAND],
                        f1t[cb][:, hl, w0 : w0 + M],
                        f2t[cb][:, hl, w0 : w0 + NBAND],
                        start=(cb == 0),
                        stop=(cb == 1),
                    )
            # psum (blk, n) -> band[:, blk, n, h0+hl], strided over n
            nc.scalar.activation(
                band[:, :, :, h0 + hl],
                ps.rearrange("p (b n) -> p b n", b=NB),
                mybir.ActivationFunctionType.Copy,
                scale=1.0 / C,
            )

    if skip_compute or skip_gather:
        return

    # One contiguous dump of all bands to flat-addressed HBM scratch.
    scratch = dpool.tile([M, NB, NBAND, H], F32, tag="scr")
    nc.sync.dma_start(
        out=scratch.rearrange("p b n h -> p (b n h)"),
        in_=band.rearrange("p b n h -> p (b n h)"),
    )

    # Diagonal gathers (96-element h-runs), PE transpose, contiguous out.
    sc_p = NB * NBAND * H  # 34560: partition-row length in scratch
    with nc.allow_non_contiguous_dma("banded diagonal gather"):
        for i in range(NS):
            osb = opool.tile([H, W], F32, tag="osb")
            for blk in range(NB):
                g = gpool.tile([M, H], F32, tag="g")
                src = bass.AP(
                    tensor=scratch.tensor,
                    offset=scratch.offset + blk * NBAND * H + i * H,
                    ap=[[sc_p + H, M], [1, H]],
                )
                nc.sync.dma_start(out=g, in_=src)
                tp = pspool.tile([H, M], F32, tag="tp", bufs=2)
                nc.tensor.transpose(tp, g, ident)
                nc.scalar.copy(out=osb[:, blk * M : (blk + 1) * M], in_=tp)
            nc.sync.dma_start(out=out.ap()[i], in_=osb)


def kernel(feat1: np.ndarray, feat2: np.ndarray) -> np.ndarray:
    if "nc" not in _cache:
        _cache["nc"] = _build()
    nc = _cache["nc"]
    feat1 = np.ascontiguousarray(feat1, dtype=np.float32)
    feat2 = np.ascontiguousarray(feat2, dtype=np.float32)
    in_maps = [{"f1": feat1[b], "f2": feat2[b]} for b in range(B)]
    res = run_bass_kernel_spmd(nc, in_maps, core_ids=list(range(B)))
    return np.stack([res.results[b]["out"] for b in range(B)], axis=0)
